# revision 15
# baseline (speedup 1.0000x reference)
"""MoE group-limited routing gate (DeepSeek-style) on 8 Trainium2 NeuronCores.

Computation (per token t over E=256 experts, D=7168 features):
    logits = x @ weight.T                      [T, E]
    group-limited top-k: 8 groups of 32 experts, keep top-4 groups by
    group-max, then top-8 experts among kept groups.
    weights = sigmoid(logits[sel]) normalized to sum 1, * 2.5
Returns (weights [T,8] f32, indices [T,8] int32) like the reference.

Strategy: data-parallel over tokens, 2048 tokens/core, gate weight
replicated.  x is pre-transposed on host to [D, T] so the contraction dim
lands on SBUF partitions.  Matmul precision options:
  - "fp16_fp8dr" (default): logits = xh@wh (fp16 full-rate pass) +
    2^-15 * (xl8@wh8 + x8@wl8) where the two small cross terms are packed
    into a SINGLE fp8 DoubleRow pass (the PE computes
    lhsT[:,0].T@rhs[:,0] + lhsT[:,1].T@rhs[:,1] per matmul).  The cross
    terms are each ~2^-12 of the logit, so 4-bit-mantissa fp8 operands
    keep total logit error ~2e-5 (idx rel-err ~6e-3, CPU-validated).
    2.0 fp16-pass-equivalents of PE time vs fp16x3's 3.0.
  - "fp16x3": x and w split into fp16 (hi, lo*2^11) pairs;
    logits = hi@hi + 2^-11*(hi@lo2 + lo2@hi).  Exact to ~1e-6 but 3
    full-rate passes.
Top-k uses the DVE native max/max_index (top-8 sorted) instructions; the
group top-4 uses a threshold trick (4th-largest group-max) since sigmoid
is monotone and masking is additive on logits.
"""

import numpy as np
from contextlib import ExitStack

import concourse.bacc as bacc
import concourse.tile as tile
from concourse import mybir
from concourse.bass_utils import run_bass_kernel_spmd

N_CORES = 8
T_FULL = 16384
D = 7168
E = 256
G = 8            # expert groups
EPG = E // G     # experts per group = 32
TOPK = 8
TOPK_GROUPS = 4
ROUTE_SCALE = 2.5

P = 128
T = T_FULL // N_CORES       # 2048 tokens per core
KC = D // P                 # 56 contraction chunks
TB = 256                    # tokens per block
NB = T // TB                # 8 blocks
TPB = TB // P               # 2 token-tiles per block
KQ = 4                      # x DMA splits per block (finer-grained deps)
KCQ = KC // KQ              # 14 k-chunks per split
WQ = 8                      # weight DMA splits
WCQ = KC // WQ              # 7 k-chunks per split
NEG = -1.0e30
LO_SCALE = 2.0 ** 11        # fp16x3: host scales the fp16 lo term by this
LO8_SCALE = 2.0 ** 15       # fp16_fp8dr: host scales both fp8 lo terms by this
PRECISION = "fp16_fp8dr"    # "fp16_fp8dr" | "fp16x3"

_CACHE = {}


def _emit_topk(nc, sc_pool, out_pool, scores, wout, iout, t0, acc=None):
    """Group-limited top-k + normalize on a [128, 256] f32 logits tile.

    If acc=(wacc, iacc, ti) is given, results land in SBUF accumulators at
    tile index ti (batched output DMA at the end); otherwise each tile DMAs
    its own [128,8] results (32 tiny SWDGE writes — ~1us tail each)."""
    f32 = mybir.dt.float32
    scores_g = scores.rearrange("p (g e) -> p g e", g=G)
    glog = sc_pool.tile([P, G], f32)
    nc.vector.reduce_max(out=glog, in_=scores_g, axis=mybir.AxisListType.X)
    gsort = sc_pool.tile([P, G], f32)
    nc.vector.max(out=gsort, in_=glog)
    # additive mask: 0 for kept groups (>= 4th-largest), -1e30 otherwise
    maskadd = sc_pool.tile([P, G], f32)
    nc.vector.tensor_scalar(
        out=maskadd,
        in0=glog,
        scalar1=gsort[:, TOPK_GROUPS - 1:TOPK_GROUPS],
        scalar2=NEG,
        op0=mybir.AluOpType.is_lt,
        op1=mybir.AluOpType.mult,
    )
    masked = sc_pool.tile([P, E], f32)
    nc.vector.tensor_add(
        masked.rearrange("p (g e) -> p g e", g=G),
        scores_g,
        maskadd.to_broadcast([P, G, EPG]),
    )
    top8 = sc_pool.tile([P, TOPK], f32)
    nc.vector.max(out=top8, in_=masked)
    if acc is not None:
        wacc, iacc, ti = acc
        idx_out = iacc[:, ti, :]
        w_out_ap = wacc[:, ti, :]
    else:
        idx_out = out_pool.tile([P, TOPK], mybir.dt.uint32)
        w_out_ap = out_pool.tile([P, TOPK], f32)
    nc.vector.max_index(out=idx_out, in_max=top8, in_values=masked)
    sig = sc_pool.tile([P, TOPK], f32)
    nc.scalar.activation(
        out=sig, in_=top8, func=mybir.ActivationFunctionType.Sigmoid
    )
    ssum = sc_pool.tile([P, 1], f32)
    nc.vector.reduce_sum(out=ssum, in_=sig, axis=mybir.AxisListType.X)
    rec = sc_pool.tile([P, 1], f32)
    nc.vector.reciprocal(out=rec, in_=ssum)
    nc.vector.tensor_scalar(
        out=w_out_ap,
        in0=sig,
        scalar1=rec[:, 0:1],
        scalar2=ROUTE_SCALE,
        op0=mybir.AluOpType.mult,
        op1=mybir.AluOpType.mult,
    )
    if acc is None:
        # outputs ride the SWDGE ring so the tiny writes never stall the
        # HWDGE ring that streams x
        nc.gpsimd.dma_start(out=wout[t0:t0 + P, :], in_=w_out_ap)
        nc.gpsimd.dma_start(out=iout[t0:t0 + P, :], in_=idx_out)


def _build_fp16_fp8dr():
    """Main fp16 pass + single fp8 DoubleRow pass for both cross terms.

    Per 128-token tile: psumA = xh@wh (56 fp16 matmuls); psumB =
    2^15*(xl@wh + x@wl) (56 DoubleRow fp8 matmuls, each contracting one
    128-feature chunk for BOTH packed terms); scores = psumA + 2^-15*psumB.

    HBM traffic is 3B per x element: xh (fp16) + xl8 (fp8); the third
    operand x8 = fp8(xh) is cast on the otherwise-idle Pool engine into
    the second half of the DoubleRow lhsT tile.

    DRAM layouts are chosen so every DMA's contiguous run is exactly 512B
    (runs <512B pay a 2x DMA penalty): xh [D,T] f16 block slice -> 256*2B;
    xl8 k-paired [NB,KC/2,P,2,TB] -> (2,256)*1B both src and dst; wh
    [D,E] -> 512B; wi8 [D,2,E] -> 512B.
    """
    nc = bacc.Bacc("TRN2", target_bir_lowering=False, debug=False, num_devices=N_CORES)
    f32 = mybir.dt.float32
    f16 = mybir.dt.float16
    f8 = mybir.dt.float8e4
    xh = nc.dram_tensor("xh", [D, T], f16, kind="ExternalInput").ap()
    xl8 = nc.dram_tensor("xl8", [NB, KC // 2, P, 2, TB], f8, kind="ExternalInput").ap()
    wh = nc.dram_tensor("wh", [D, E], f16, kind="ExternalInput").ap()
    wi8 = nc.dram_tensor("wi8", [D, 2, E], f8, kind="ExternalInput").ap()
    wout = nc.dram_tensor("w_out", [T, TOPK], f32, kind="ExternalOutput").ap()
    iout = nc.dram_tensor("i_out", [T, TOPK], mybir.dt.uint32, kind="ExternalOutput").ap()

    xh_r = xh.rearrange("(k p) t -> p k t", p=P)
    # [p, nb, pair, two, t] view of the k-paired layout (k = pair*2 + two)
    xl8_r = xl8.rearrange("nb pair p two t -> p nb pair two t")
    wh_r = wh.rearrange("(k p) e -> p k e", p=P)
    wi8_r = wi8.rearrange("(k p) two e -> p k two e", p=P)

    with tile.TileContext(nc) as tc, ExitStack() as ctx:
        wt_pool = ctx.enter_context(tc.tile_pool(name="wt", bufs=1))
        xt_pool = ctx.enter_context(tc.tile_pool(name="xt", bufs=2))
        psA_pool = ctx.enter_context(tc.tile_pool(name="psA", bufs=4, space="PSUM"))
        psB_pool = ctx.enter_context(tc.tile_pool(name="psB", bufs=4, space="PSUM"))
        sc_pool = ctx.enter_context(tc.tile_pool(name="scratch", bufs=3))
        out_pool = ctx.enter_context(tc.tile_pool(name="outs", bufs=1))

        NT = NB * TPB  # 16 token tiles per core
        wacc = out_pool.tile([P, NT, TOPK], f32, tag="wacc")
        iacc = out_pool.tile([P, NT, TOPK], mybir.dt.uint32, tag="iacc")

        wh_sb, wi8_sb = [], []

        def load_wh(q):
            wtile = wt_pool.tile([P, WCQ, E], f16, tag=f"wh{q}")
            nc.sync.dma_start(out=wtile, in_=wh_r[:, q * WCQ:(q + 1) * WCQ, :])
            wh_sb.append(wtile)

        def load_wi8(q):
            wtile = wt_pool.tile([P, WCQ, 2, E], f8, tag=f"wi8{q}")
            nc.sync.dma_start(out=wtile, in_=wi8_r[:, q * WCQ:(q + 1) * WCQ, :, :])
            wi8_sb.append(wtile)

        def load_xh_q(b, q):
            xtile = xt_pool.tile([P, KCQ, TB], f16, tag=f"xh{q}")
            nc.sync.dma_start(
                out=xtile, in_=xh_r[:, q * KCQ:(q + 1) * KCQ, b * TB:(b + 1) * TB]
            )
            return xtile

        def load_xi8_q(b, q, xh_tile):
            """xi8 tile [P, 2, KCQ, TB]: [:,0]=xl8 (DMA), [:,1]=fp8(xh)
            (Pool cast — keeps 14.7MB/core off the HBM stream)."""
            xtile = xt_pool.tile([P, 2, KCQ, TB], f8, tag=f"xi8{q}")
            npair = KCQ // 2
            nc.sync.dma_start(
                out=xtile[:, 0, :, :].rearrange("p (pair two) t -> p pair two t", two=2),
                in_=xl8_r[:, b, q * npair:(q + 1) * npair, :, :],
            )
            nc.gpsimd.tensor_scalar(
                out=xtile[:, 1, :, :],
                in0=xh_tile,
                scalar1=1.0,
                scalar2=None,
                op0=mybir.AluOpType.mult,
            )
            return xtile

        def load_block(b):
            xh_q, xi8_q = [], []
            for q in range(KQ):
                xh_q.append(load_xh_q(b, q))
            for q in range(KQ):
                xi8_q.append(load_xi8_q(b, q, xh_q[q]))
            return xh_q, xi8_q

        # DMA emission order == HWDGE arrival order (FIFO ring).  Phase 1:
        # wh eighths interleaved with block-0 xh quarters so the fp16 A-pass
        # starts within a few us.  Phase 2: wi8 + block-0 xl8 for the B-pass.
        # Then blocks 1..7 stream (xh quarters, then xl8 quarters).
        xh_blocks, xi8_blocks = {}, {}
        xh0, xi80 = [], []
        for q in range(KQ):
            load_wh(2 * q)
            load_wh(2 * q + 1)
            xh0.append(load_xh_q(0, q))
        for q in range(KQ):
            load_wi8(2 * q)
            load_wi8(2 * q + 1)
            xi80.append(load_xi8_q(0, q, xh0[q]))
        xh_blocks[0], xi8_blocks[0] = xh0, xi80

        for b in range(NB):
            if b not in xh_blocks:
                xh_blocks[b], xi8_blocks[b] = load_block(b)
            xh_q = xh_blocks.pop(b)
            xi8_q = xi8_blocks.pop(b)
            for j in range(TPB):
                js = slice(j * P, (j + 1) * P)
                psumA = psA_pool.tile([P, E], f32)
                for k in range(KC):
                    nc.tensor.matmul(
                        psumA,
                        xh_q[k // KCQ][:, k % KCQ, js],
                        wh_sb[k // WCQ][:, k % WCQ, :],
                        start=(k == 0),
                        stop=(k == KC - 1),
                    )
                psumB = psB_pool.tile([P, E], f32)
                for k in range(KC):
                    nc.tensor.matmul(
                        psumB,
                        xi8_q[k // KCQ][:, :, k % KCQ, js],
                        wi8_sb[k // WCQ][:, k % WCQ, :, :],
                        start=(k == 0),
                        stop=(k == KC - 1),
                        perf_mode=mybir.MatmulPerfMode.DoubleRow,
                    )
                scores = sc_pool.tile([P, E], f32)
                nc.scalar.activation(
                    out=scores,
                    in_=psumB,
                    func=mybir.ActivationFunctionType.Copy,
                    scale=1.0 / LO8_SCALE,
                )
                nc.vector.tensor_add(scores, scores, psumA)
                _emit_topk(
                    nc, sc_pool, out_pool, scores, wout, iout, b * TB + j * P,
                    acc=(wacc, iacc, b * TPB + j),
                )

        # two batched output writes instead of 32 tiny per-tile DMAs; they
        # ride the SP/ACT HWDGE rings so the preps run in parallel (the
        # SWDGE descriptor build is ~1.7us each on Pool, serialized)
        nc.sync.dma_start(out=wout.rearrange("(i p) k -> p i k", p=P), in_=wacc)
        nc.scalar.dma_start(out=iout.rearrange("(i p) k -> p i k", p=P), in_=iacc)
    nc.compile()
    return nc


def _build_fp16x3():
    nc = bacc.Bacc("TRN2", target_bir_lowering=False, debug=False, num_devices=N_CORES)
    f32 = mybir.dt.float32
    f16 = mybir.dt.float16
    xh = nc.dram_tensor("xh", [D, T], f16, kind="ExternalInput").ap()
    xl = nc.dram_tensor("xl", [D, T], f16, kind="ExternalInput").ap()
    wh = nc.dram_tensor("wh", [D, E], f16, kind="ExternalInput").ap()
    wl = nc.dram_tensor("wl", [D, E], f16, kind="ExternalInput").ap()
    wout = nc.dram_tensor("w_out", [T, TOPK], f32, kind="ExternalOutput").ap()
    iout = nc.dram_tensor("i_out", [T, TOPK], mybir.dt.uint32, kind="ExternalOutput").ap()

    xh_r = xh.rearrange("(k p) t -> p k t", p=P)
    xl_r = xl.rearrange("(k p) t -> p k t", p=P)
    wh_r = wh.rearrange("(k p) e -> p k e", p=P)
    wl_r = wl.rearrange("(k p) e -> p k e", p=P)

    with tile.TileContext(nc) as tc, ExitStack() as ctx:
        wt_pool = ctx.enter_context(tc.tile_pool(name="wt", bufs=1))
        xt_pool = ctx.enter_context(tc.tile_pool(name="xt", bufs=2))
        # 4+4 slots = all 8 PSUM banks: block b's accumulators coexist with
        # block b-1's (whose xh@wl half is deferred one block, see below)
        psA_pool = ctx.enter_context(tc.tile_pool(name="psA", bufs=4, space="PSUM"))
        psB_pool = ctx.enter_context(tc.tile_pool(name="psB", bufs=4, space="PSUM"))
        sc_pool = ctx.enter_context(tc.tile_pool(name="scratch", bufs=3))
        out_pool = ctx.enter_context(tc.tile_pool(name="outs", bufs=4))

        def load_w(q, which):
            src, lst, tag = (
                (wh_r, wh_sb, f"wh{q}") if which == "h" else (wl_r, wl_sb, f"wl{q}")
            )
            wtile = wt_pool.tile([P, WCQ, E], f16, tag=tag)
            nc.sync.dma_start(out=wtile, in_=src[:, q * WCQ:(q + 1) * WCQ, :])
            lst.append(wtile)

        def load_x_block(b):
            xh_q, xl_q = [], []
            t_lo, t_hi = b * TB, (b + 1) * TB
            for q in range(KQ):
                xtile = xt_pool.tile([P, KCQ, TB], f16, tag=f"xh{q}")
                nc.sync.dma_start(
                    out=xtile, in_=xh_r[:, q * KCQ:(q + 1) * KCQ, t_lo:t_hi]
                )
                xh_q.append(xtile)
                ltile = xt_pool.tile([P, KCQ, TB], f16, tag=f"xl{q}")
                nc.sync.dma_start(
                    out=ltile, in_=xl_r[:, q * KCQ:(q + 1) * KCQ, t_lo:t_hi]
                )
                xl_q.append(ltile)
            return xh_q, xl_q

        wh_sb, wl_sb = [], []
        xh0, xl0 = [], []
        t_hi0 = TB
        for q in range(KQ):
            load_w(2 * q, "h")
            load_w(2 * q + 1, "h")
            xtile = xt_pool.tile([P, KCQ, TB], f16, tag=f"xh{q}")
            nc.sync.dma_start(out=xtile, in_=xh_r[:, q * KCQ:(q + 1) * KCQ, 0:t_hi0])
            xh0.append(xtile)
        for q in range(KQ):
            ltile = xt_pool.tile([P, KCQ, TB], f16, tag=f"xl{q}")
            nc.sync.dma_start(out=ltile, in_=xl_r[:, q * KCQ:(q + 1) * KCQ, 0:t_hi0])
            xl0.append(ltile)
        for q in range(WQ):
            load_w(q, "l")
        blocks = {0: (xh0, xl0)}

        def flush(state):
            bb, xh_q, psA_list, psB_list = state
            for j in range(TPB):
                js = slice(j * P, (j + 1) * P)
                psumB = psB_list[j]
                for k in range(KC):
                    nc.tensor.matmul(
                        psumB,
                        xh_q[k // KCQ][:, k % KCQ, js],
                        wl_sb[k // WCQ][:, k % WCQ, :],
                        start=False,
                        stop=(k == KC - 1),
                    )
                scores = sc_pool.tile([P, E], f32)
                nc.scalar.activation(
                    out=scores,
                    in_=psumB,
                    func=mybir.ActivationFunctionType.Copy,
                    scale=1.0 / LO_SCALE,
                )
                nc.vector.tensor_add(scores, scores, psA_list[j])
                _emit_topk(nc, sc_pool, out_pool, scores, wout, iout, bb * TB + j * P)

        pending = None
        for b in range(NB):
            if b not in blocks:
                blocks[b] = load_x_block(b)
            xh_q, xl_q = blocks.pop(b)
            if b == 0:
                psA_list, psB_list = [], []
                for j in range(TPB):
                    js = slice(j * P, (j + 1) * P)
                    psumA = psA_pool.tile([P, E], f32)
                    for k in range(KC):
                        nc.tensor.matmul(
                            psumA,
                            xh_q[k // KCQ][:, k % KCQ, js],
                            wh_sb[k // WCQ][:, k % WCQ, :],
                            start=(k == 0),
                            stop=(k == KC - 1),
                        )
                    psA_list.append(psumA)
                for j in range(TPB):
                    js = slice(j * P, (j + 1) * P)
                    psumB = psB_pool.tile([P, E], f32)
                    for k in range(KC):
                        nc.tensor.matmul(
                            psumB,
                            xl_q[k // KCQ][:, k % KCQ, js],
                            wh_sb[k // WCQ][:, k % WCQ, :],
                            start=(k == 0),
                            stop=False,
                        )
                    psB_list.append(psumB)
                pending = (b, xh_q, psA_list, psB_list)
                continue
            for j in range(TPB):
                js = slice(j * P, (j + 1) * P)
                psumA = psA_pool.tile([P, E], f32)
                for k in range(KC):
                    nc.tensor.matmul(
                        psumA,
                        xh_q[k // KCQ][:, k % KCQ, js],
                        wh_sb[k // WCQ][:, k % WCQ, :],
                        start=(k == 0),
                        stop=(k == KC - 1),
                    )
                if pending is not None:
                    flush(pending)
                    pending = None
                psumB = psB_pool.tile([P, E], f32)
                for i in range(2 * KC):
                    k = i % KC
                    if i < KC:
                        lhsT = xl_q[k // KCQ][:, k % KCQ, js]
                        rhs = wh_sb[k // WCQ][:, k % WCQ, :]
                    else:
                        lhsT = xh_q[k // KCQ][:, k % KCQ, js]
                        rhs = wl_sb[k // WCQ][:, k % WCQ, :]
                    nc.tensor.matmul(
                        psumB, lhsT, rhs, start=(i == 0), stop=(i == 2 * KC - 1)
                    )
                scores = sc_pool.tile([P, E], f32)
                nc.scalar.activation(
                    out=scores,
                    in_=psumB,
                    func=mybir.ActivationFunctionType.Copy,
                    scale=1.0 / LO_SCALE,
                )
                nc.vector.tensor_add(scores, scores, psumA)
                _emit_topk(nc, sc_pool, out_pool, scores, wout, iout, b * TB + j * P)
    nc.compile()
    return nc


def _get_program(precision):
    key = f"nc_{precision}"
    if key not in _CACHE:
        _CACHE[key] = (
            _build_fp16_fp8dr() if precision == "fp16_fp8dr" else _build_fp16x3()
        )
    return _CACHE[key]


def _split_f16(a):
    hi = a.astype(np.float16)
    lo = ((a - hi.astype(np.float32)) * np.float32(LO_SCALE)).astype(np.float16)
    return hi, lo


def _prep_fp16_fp8dr(xt_full, wt_host):
    """Host-side operand prep for the fp16+fp8DR kernel.

    xt_full: [D, T_FULL] f32 (x transposed); wt_host: [D, E] f32.
    Returns per-core input dicts.  xl8 ships k-paired/blocked
    [NB, KC//2, P, 2, TB] so both src and dst DMA runs are 512B; x8 is
    cast from xh on device.
    """
    import ml_dtypes

    f8 = ml_dtypes.float8_e4m3
    S = np.float32(LO8_SCALE)
    xh_full = xt_full.astype(np.float16)                       # [D, T_FULL]
    xl8_full = ((xt_full - xh_full.astype(np.float32)) * S).astype(f8)
    wh_host = wt_host.astype(np.float16)                       # [D, E]
    wh8 = wh_host.astype(f8)
    wl8 = ((wt_host - wh_host.astype(np.float32)) * S).astype(f8)
    wi8_host = np.ascontiguousarray(np.stack([wh8, wl8], axis=1))  # [D, 2, E]

    in_maps = []
    for c in range(N_CORES):
        sl = slice(c * T, (c + 1) * T)
        # [D, T] -> [KC//2, 2, P, NB, TB] -> [NB, KC//2, P, 2, TB]
        xl8_c = xl8_full[:, sl].reshape(KC // 2, 2, P, NB, TB)
        xl8_c = np.ascontiguousarray(xl8_c.transpose(3, 0, 2, 1, 4))
        in_maps.append(
            {
                "xh": np.ascontiguousarray(xh_full[:, sl]),
                "xl8": xl8_c,
                "wh": wh_host,
                "wi8": wi8_host,
            }
        )
    return in_maps


def kernel(x: np.ndarray, weight: np.ndarray, _trace: bool = False, **_kw):
    x = np.asarray(x, dtype=np.float32)
    weight = np.asarray(weight, dtype=np.float32)
    assert x.shape == (T_FULL, D) and weight.shape == (E, D)

    nc = _get_program(PRECISION)
    xt_full = np.ascontiguousarray(x.T)              # [D, T_FULL]
    wt_host = np.ascontiguousarray(weight.T)         # [D, E]
    if PRECISION == "fp16_fp8dr":
        in_maps = _prep_fp16_fp8dr(xt_full, wt_host)
    else:
        xh_full, xl_full = _split_f16(xt_full)
        wh_host, wl_host = _split_f16(wt_host)
        in_maps = [
            {
                "xh": np.ascontiguousarray(xh_full[:, c * T:(c + 1) * T]),
                "xl": np.ascontiguousarray(xl_full[:, c * T:(c + 1) * T]),
                "wh": wh_host,
                "wl": wl_host,
            }
            for c in range(N_CORES)
        ]
    if _trace:
        import prof

        results, exec_time_ns, percore, neff_dir = prof.profiled_run(
            nc, in_maps, core_ids=list(range(N_CORES))
        )
        _CACHE["last_result"] = {
            "exec_time_ns": exec_time_ns,
            "percore": percore,
            "neff_dir": neff_dir,
        }
    else:
        res = run_bass_kernel_spmd(nc, in_maps, core_ids=list(range(N_CORES)))
        results = res.results
    w_full = np.concatenate([results[c]["w_out"] for c in range(N_CORES)], axis=0)
    i_full = np.concatenate(
        [results[c]["i_out"].astype(np.int32) for c in range(N_CORES)], axis=0
    )
    return w_full, i_full


# revision 16
# speedup vs baseline: 1.1509x; 1.1509x over previous
"""MoE group-limited routing gate (DeepSeek-style) on 8 Trainium2 NeuronCores.

Computation (per token t over E=256 experts, D=7168 features):
    logits = x @ weight.T                      [T, E]
    group-limited top-k: 8 groups of 32 experts, keep top-4 groups by
    group-max, then top-8 experts among kept groups.
    weights = sigmoid(logits[sel]) normalized to sum 1, * 2.5
Returns (weights [T,8] f32, indices [T,8] int32) like the reference.

Strategy: data-parallel over tokens, 2048 tokens/core, gate weight
replicated.  x is pre-transposed on host to [D, T] so the contraction dim
lands on SBUF partitions.  Matmul precision options:
  - "fp16_fp8dr" (default): logits = xh@wh (fp16 full-rate pass) +
    2^-15 * (xl8@wh8 + x8@wl8) where the two small cross terms are packed
    into a SINGLE fp8 DoubleRow pass (the PE computes
    lhsT[:,0].T@rhs[:,0] + lhsT[:,1].T@rhs[:,1] per matmul).  The cross
    terms are each ~2^-12 of the logit, so 4-bit-mantissa fp8 operands
    keep total logit error ~2e-5 (idx rel-err ~6e-3, CPU-validated).
    2.0 fp16-pass-equivalents of PE time vs fp16x3's 3.0.
  - "fp16x3": x and w split into fp16 (hi, lo*2^11) pairs;
    logits = hi@hi + 2^-11*(hi@lo2 + lo2@hi).  Exact to ~1e-6 but 3
    full-rate passes.
Top-k uses the DVE native max/max_index (top-8 sorted) instructions; the
group top-4 uses a threshold trick (4th-largest group-max) since sigmoid
is monotone and masking is additive on logits.
"""

import numpy as np
from contextlib import ExitStack

import concourse.bacc as bacc
import concourse.tile as tile
from concourse import mybir
from concourse.bass_utils import run_bass_kernel_spmd

N_CORES = 8
T_FULL = 16384
D = 7168
E = 256
G = 8            # expert groups
EPG = E // G     # experts per group = 32
TOPK = 8
TOPK_GROUPS = 4
ROUTE_SCALE = 2.5

P = 128
T = T_FULL // N_CORES       # 2048 tokens per core
KC = D // P                 # 56 contraction chunks
TB = 256                    # tokens per block
NB = T // TB                # 8 blocks
TPB = TB // P               # 2 token-tiles per block
KQ = 4                      # x DMA splits per block (finer-grained deps)
KCQ = KC // KQ              # 14 k-chunks per split
WQ = 8                      # weight DMA splits
WCQ = KC // WQ              # 7 k-chunks per split
NEG = -1.0e30
LO_SCALE = 2.0 ** 11        # fp16x3: host scales the fp16 lo term by this
LO8_SCALE = 2.0 ** 15       # fp16_fp8dr: host scales both fp8 lo terms by this
PRECISION = "fp16_fp8dr"    # "fp16_fp8dr" | "fp16x3"

_CACHE = {}


def _emit_topk(nc, sc_pool, out_pool, scores, wout, iout, t0, acc=None):
    """Group-limited top-k + normalize on a [128, 256] f32 logits tile.

    If acc=(wacc, iacc, ti) is given, results land in SBUF accumulators at
    tile index ti (batched output DMA at the end); otherwise each tile DMAs
    its own [128,8] results (32 tiny SWDGE writes — ~1us tail each)."""
    f32 = mybir.dt.float32
    scores_g = scores.rearrange("p (g e) -> p g e", g=G)
    glog = sc_pool.tile([P, G], f32)
    nc.vector.reduce_max(out=glog, in_=scores_g, axis=mybir.AxisListType.X)
    gsort = sc_pool.tile([P, G], f32)
    nc.vector.max(out=gsort, in_=glog)
    # additive mask: 0 for kept groups (>= 4th-largest), -1e30 otherwise
    maskadd = sc_pool.tile([P, G], f32)
    nc.vector.tensor_scalar(
        out=maskadd,
        in0=glog,
        scalar1=gsort[:, TOPK_GROUPS - 1:TOPK_GROUPS],
        scalar2=NEG,
        op0=mybir.AluOpType.is_lt,
        op1=mybir.AluOpType.mult,
    )
    masked = sc_pool.tile([P, E], f32)
    nc.vector.tensor_add(
        masked.rearrange("p (g e) -> p g e", g=G),
        scores_g,
        maskadd.to_broadcast([P, G, EPG]),
    )
    top8 = sc_pool.tile([P, TOPK], f32)
    nc.vector.max(out=top8, in_=masked)
    if acc is not None:
        wacc, iacc, ti = acc
        idx_out = iacc[:, ti, :]
        w_out_ap = wacc[:, ti, :]
    else:
        idx_out = out_pool.tile([P, TOPK], mybir.dt.uint32)
        w_out_ap = out_pool.tile([P, TOPK], f32)
    nc.vector.max_index(out=idx_out, in_max=top8, in_values=masked)
    sig = sc_pool.tile([P, TOPK], f32)
    nc.scalar.activation(
        out=sig, in_=top8, func=mybir.ActivationFunctionType.Sigmoid
    )
    ssum = sc_pool.tile([P, 1], f32)
    nc.vector.reduce_sum(out=ssum, in_=sig, axis=mybir.AxisListType.X)
    rec = sc_pool.tile([P, 1], f32)
    nc.vector.reciprocal(out=rec, in_=ssum)
    nc.vector.tensor_scalar(
        out=w_out_ap,
        in0=sig,
        scalar1=rec[:, 0:1],
        scalar2=ROUTE_SCALE,
        op0=mybir.AluOpType.mult,
        op1=mybir.AluOpType.mult,
    )
    if acc is None:
        # outputs ride the SWDGE ring so the tiny writes never stall the
        # HWDGE ring that streams x
        nc.gpsimd.dma_start(out=wout[t0:t0 + P, :], in_=w_out_ap)
        nc.gpsimd.dma_start(out=iout[t0:t0 + P, :], in_=idx_out)


def _build_fp16_fp8dr():
    """Main fp16 pass + single fp8 DoubleRow pass for both cross terms.

    Per 128-token tile: psumA = xh@wh (56 fp16 matmuls); psumB =
    2^15*(xl@wh + x@wl) (56 DoubleRow fp8 matmuls, each contracting one
    128-feature chunk for BOTH packed terms); scores = psumA + 2^-15*psumB.

    HBM traffic is 3B per x element: xh (fp16) + xl8 (fp8); the third
    operand x8 = fp8(xh) is cast on the otherwise-idle Pool engine into
    the second half of the DoubleRow lhsT tile.

    DRAM layouts are chosen so every DMA's contiguous run is exactly 512B
    (runs <512B pay a 2x DMA penalty): xh [D,T] f16 block slice -> 256*2B;
    xl8 k-paired [NB,KC/2,P,2,TB] -> (2,256)*1B both src and dst; wh
    [D,E] -> 512B; wi8 [D,2,E] -> 512B.
    """
    nc = bacc.Bacc("TRN2", target_bir_lowering=False, debug=False, num_devices=N_CORES)
    f32 = mybir.dt.float32
    f16 = mybir.dt.float16
    f8 = mybir.dt.float8e4
    xh = nc.dram_tensor("xh", [D, T], f16, kind="ExternalInput").ap()
    xl8 = nc.dram_tensor("xl8", [NB, KC // 2, P, 2, TB], f8, kind="ExternalInput").ap()
    wh = nc.dram_tensor("wh", [D, E], f16, kind="ExternalInput").ap()
    wi8 = nc.dram_tensor("wi8", [D, 2, E], f8, kind="ExternalInput").ap()
    wout = nc.dram_tensor("w_out", [T, TOPK], f32, kind="ExternalOutput").ap()
    iout = nc.dram_tensor("i_out", [T, TOPK], mybir.dt.uint32, kind="ExternalOutput").ap()

    xh_r = xh.rearrange("(k p) t -> p k t", p=P)
    # [p, nb, pair, two, t] view of the k-paired layout (k = pair*2 + two)
    xl8_r = xl8.rearrange("nb pair p two t -> p nb pair two t")
    wh_r = wh.rearrange("(k p) e -> p k e", p=P)
    wi8_r = wi8.rearrange("(k p) two e -> p k two e", p=P)

    with tile.TileContext(nc) as tc, ExitStack() as ctx:
        wt_pool = ctx.enter_context(tc.tile_pool(name="wt", bufs=1))
        xt_pool = ctx.enter_context(tc.tile_pool(name="xt", bufs=2))
        psA_pool = ctx.enter_context(tc.tile_pool(name="psA", bufs=4, space="PSUM"))
        psB_pool = ctx.enter_context(tc.tile_pool(name="psB", bufs=4, space="PSUM"))
        sc_pool = ctx.enter_context(tc.tile_pool(name="scratch", bufs=3))
        out_pool = ctx.enter_context(tc.tile_pool(name="outs", bufs=1))

        NT = NB * TPB  # 16 token tiles per core
        wacc = out_pool.tile([P, NT, TOPK], f32, tag="wacc")
        iacc = out_pool.tile([P, NT, TOPK], mybir.dt.uint32, tag="iacc")

        wh_sb, wi8_sb = [], []

        def load_wh(q):
            wtile = wt_pool.tile([P, WCQ, E], f16, tag=f"wh{q}")
            nc.sync.dma_start(out=wtile, in_=wh_r[:, q * WCQ:(q + 1) * WCQ, :])
            wh_sb.append(wtile)

        def load_wi8(q):
            wtile = wt_pool.tile([P, WCQ, 2, E], f8, tag=f"wi8{q}")
            nc.sync.dma_start(out=wtile, in_=wi8_r[:, q * WCQ:(q + 1) * WCQ, :, :])
            wi8_sb.append(wtile)

        def load_xh_q(b, q):
            xtile = xt_pool.tile([P, KCQ, TB], f16, tag=f"xh{q}")
            nc.sync.dma_start(
                out=xtile, in_=xh_r[:, q * KCQ:(q + 1) * KCQ, b * TB:(b + 1) * TB]
            )
            return xtile

        def load_xi8_q(b, q, xh_tile):
            """xi8 tile [P, 2, KCQ, TB]: [:,0]=xl8 (DMA), [:,1]=fp8(xh)
            (engine cast — keeps 14.7MB/core off the HBM stream).  The cast
            is byte-paced (~5us per quarter), so quarters alternate between
            the Pool and DVE engines to stay off the critical path."""
            xtile = xt_pool.tile([P, 2, KCQ, TB], f8, tag=f"xi8{q}")
            npair = KCQ // 2
            nc.sync.dma_start(
                out=xtile[:, 0, :, :].rearrange("p (pair two) t -> p pair two t", two=2),
                in_=xl8_r[:, b, q * npair:(q + 1) * npair, :, :],
            )
            eng = nc.gpsimd if (b * KQ + q) % 2 == 0 else nc.vector
            eng.tensor_scalar(
                out=xtile[:, 1, :, :],
                in0=xh_tile,
                scalar1=1.0,
                scalar2=None,
                op0=mybir.AluOpType.mult,
            )
            return xtile

        def load_block(b):
            xh_q, xi8_q = [], []
            for q in range(KQ):
                xh_q.append(load_xh_q(b, q))
            for q in range(KQ):
                xi8_q.append(load_xi8_q(b, q, xh_q[q]))
            return xh_q, xi8_q

        # DMA emission order == HWDGE arrival order (FIFO ring).  Phase 1:
        # wh eighths interleaved with block-0 xh quarters so the fp16 A-pass
        # starts within a few us.  Phase 2: wi8 + block-0 xl8 for the B-pass.
        # Then blocks 1..7 stream (xh quarters, then xl8 quarters).
        xh_blocks, xi8_blocks = {}, {}
        xh0, xi80 = [], []
        for q in range(KQ):
            load_wh(2 * q)
            load_wh(2 * q + 1)
            xh0.append(load_xh_q(0, q))
        for q in range(KQ):
            load_wi8(2 * q)
            load_wi8(2 * q + 1)
            xi80.append(load_xi8_q(0, q, xh0[q]))
        xh_blocks[0], xi8_blocks[0] = xh0, xi80

        for b in range(NB):
            if b not in xh_blocks:
                xh_blocks[b], xi8_blocks[b] = load_block(b)
            xh_q = xh_blocks.pop(b)
            xi8_q = xi8_blocks.pop(b)
            for j in range(TPB):
                js = slice(j * P, (j + 1) * P)
                psumA = psA_pool.tile([P, E], f32)
                for k in range(KC):
                    nc.tensor.matmul(
                        psumA,
                        xh_q[k // KCQ][:, k % KCQ, js],
                        wh_sb[k // WCQ][:, k % WCQ, :],
                        start=(k == 0),
                        stop=(k == KC - 1),
                    )
                psumB = psB_pool.tile([P, E], f32)
                for k in range(KC):
                    nc.tensor.matmul(
                        psumB,
                        xi8_q[k // KCQ][:, :, k % KCQ, js],
                        wi8_sb[k // WCQ][:, k % WCQ, :, :],
                        start=(k == 0),
                        stop=(k == KC - 1),
                        perf_mode=mybir.MatmulPerfMode.DoubleRow,
                    )
                scores = sc_pool.tile([P, E], f32)
                nc.scalar.activation(
                    out=scores,
                    in_=psumB,
                    func=mybir.ActivationFunctionType.Copy,
                    scale=1.0 / LO8_SCALE,
                )
                nc.vector.tensor_add(scores, scores, psumA)
                _emit_topk(
                    nc, sc_pool, out_pool, scores, wout, iout, b * TB + j * P,
                    acc=(wacc, iacc, b * TPB + j),
                )

        # two batched output writes instead of 32 tiny per-tile DMAs; they
        # ride the SP/ACT HWDGE rings so the preps run in parallel (the
        # SWDGE descriptor build is ~1.7us each on Pool, serialized)
        nc.sync.dma_start(out=wout.rearrange("(i p) k -> p i k", p=P), in_=wacc)
        nc.scalar.dma_start(out=iout.rearrange("(i p) k -> p i k", p=P), in_=iacc)
    nc.compile()
    return nc


def _build_fp16x3():
    nc = bacc.Bacc("TRN2", target_bir_lowering=False, debug=False, num_devices=N_CORES)
    f32 = mybir.dt.float32
    f16 = mybir.dt.float16
    xh = nc.dram_tensor("xh", [D, T], f16, kind="ExternalInput").ap()
    xl = nc.dram_tensor("xl", [D, T], f16, kind="ExternalInput").ap()
    wh = nc.dram_tensor("wh", [D, E], f16, kind="ExternalInput").ap()
    wl = nc.dram_tensor("wl", [D, E], f16, kind="ExternalInput").ap()
    wout = nc.dram_tensor("w_out", [T, TOPK], f32, kind="ExternalOutput").ap()
    iout = nc.dram_tensor("i_out", [T, TOPK], mybir.dt.uint32, kind="ExternalOutput").ap()

    xh_r = xh.rearrange("(k p) t -> p k t", p=P)
    xl_r = xl.rearrange("(k p) t -> p k t", p=P)
    wh_r = wh.rearrange("(k p) e -> p k e", p=P)
    wl_r = wl.rearrange("(k p) e -> p k e", p=P)

    with tile.TileContext(nc) as tc, ExitStack() as ctx:
        wt_pool = ctx.enter_context(tc.tile_pool(name="wt", bufs=1))
        xt_pool = ctx.enter_context(tc.tile_pool(name="xt", bufs=2))
        # 4+4 slots = all 8 PSUM banks: block b's accumulators coexist with
        # block b-1's (whose xh@wl half is deferred one block, see below)
        psA_pool = ctx.enter_context(tc.tile_pool(name="psA", bufs=4, space="PSUM"))
        psB_pool = ctx.enter_context(tc.tile_pool(name="psB", bufs=4, space="PSUM"))
        sc_pool = ctx.enter_context(tc.tile_pool(name="scratch", bufs=3))
        out_pool = ctx.enter_context(tc.tile_pool(name="outs", bufs=4))

        def load_w(q, which):
            src, lst, tag = (
                (wh_r, wh_sb, f"wh{q}") if which == "h" else (wl_r, wl_sb, f"wl{q}")
            )
            wtile = wt_pool.tile([P, WCQ, E], f16, tag=tag)
            nc.sync.dma_start(out=wtile, in_=src[:, q * WCQ:(q + 1) * WCQ, :])
            lst.append(wtile)

        def load_x_block(b):
            xh_q, xl_q = [], []
            t_lo, t_hi = b * TB, (b + 1) * TB
            for q in range(KQ):
                xtile = xt_pool.tile([P, KCQ, TB], f16, tag=f"xh{q}")
                nc.sync.dma_start(
                    out=xtile, in_=xh_r[:, q * KCQ:(q + 1) * KCQ, t_lo:t_hi]
                )
                xh_q.append(xtile)
                ltile = xt_pool.tile([P, KCQ, TB], f16, tag=f"xl{q}")
                nc.sync.dma_start(
                    out=ltile, in_=xl_r[:, q * KCQ:(q + 1) * KCQ, t_lo:t_hi]
                )
                xl_q.append(ltile)
            return xh_q, xl_q

        wh_sb, wl_sb = [], []
        xh0, xl0 = [], []
        t_hi0 = TB
        for q in range(KQ):
            load_w(2 * q, "h")
            load_w(2 * q + 1, "h")
            xtile = xt_pool.tile([P, KCQ, TB], f16, tag=f"xh{q}")
            nc.sync.dma_start(out=xtile, in_=xh_r[:, q * KCQ:(q + 1) * KCQ, 0:t_hi0])
            xh0.append(xtile)
        for q in range(KQ):
            ltile = xt_pool.tile([P, KCQ, TB], f16, tag=f"xl{q}")
            nc.sync.dma_start(out=ltile, in_=xl_r[:, q * KCQ:(q + 1) * KCQ, 0:t_hi0])
            xl0.append(ltile)
        for q in range(WQ):
            load_w(q, "l")
        blocks = {0: (xh0, xl0)}

        def flush(state):
            bb, xh_q, psA_list, psB_list = state
            for j in range(TPB):
                js = slice(j * P, (j + 1) * P)
                psumB = psB_list[j]
                for k in range(KC):
                    nc.tensor.matmul(
                        psumB,
                        xh_q[k // KCQ][:, k % KCQ, js],
                        wl_sb[k // WCQ][:, k % WCQ, :],
                        start=False,
                        stop=(k == KC - 1),
                    )
                scores = sc_pool.tile([P, E], f32)
                nc.scalar.activation(
                    out=scores,
                    in_=psumB,
                    func=mybir.ActivationFunctionType.Copy,
                    scale=1.0 / LO_SCALE,
                )
                nc.vector.tensor_add(scores, scores, psA_list[j])
                _emit_topk(nc, sc_pool, out_pool, scores, wout, iout, bb * TB + j * P)

        pending = None
        for b in range(NB):
            if b not in blocks:
                blocks[b] = load_x_block(b)
            xh_q, xl_q = blocks.pop(b)
            if b == 0:
                psA_list, psB_list = [], []
                for j in range(TPB):
                    js = slice(j * P, (j + 1) * P)
                    psumA = psA_pool.tile([P, E], f32)
                    for k in range(KC):
                        nc.tensor.matmul(
                            psumA,
                            xh_q[k // KCQ][:, k % KCQ, js],
                            wh_sb[k // WCQ][:, k % WCQ, :],
                            start=(k == 0),
                            stop=(k == KC - 1),
                        )
                    psA_list.append(psumA)
                for j in range(TPB):
                    js = slice(j * P, (j + 1) * P)
                    psumB = psB_pool.tile([P, E], f32)
                    for k in range(KC):
                        nc.tensor.matmul(
                            psumB,
                            xl_q[k // KCQ][:, k % KCQ, js],
                            wh_sb[k // WCQ][:, k % WCQ, :],
                            start=(k == 0),
                            stop=False,
                        )
                    psB_list.append(psumB)
                pending = (b, xh_q, psA_list, psB_list)
                continue
            for j in range(TPB):
                js = slice(j * P, (j + 1) * P)
                psumA = psA_pool.tile([P, E], f32)
                for k in range(KC):
                    nc.tensor.matmul(
                        psumA,
                        xh_q[k // KCQ][:, k % KCQ, js],
                        wh_sb[k // WCQ][:, k % WCQ, :],
                        start=(k == 0),
                        stop=(k == KC - 1),
                    )
                if pending is not None:
                    flush(pending)
                    pending = None
                psumB = psB_pool.tile([P, E], f32)
                for i in range(2 * KC):
                    k = i % KC
                    if i < KC:
                        lhsT = xl_q[k // KCQ][:, k % KCQ, js]
                        rhs = wh_sb[k // WCQ][:, k % WCQ, :]
                    else:
                        lhsT = xh_q[k // KCQ][:, k % KCQ, js]
                        rhs = wl_sb[k // WCQ][:, k % WCQ, :]
                    nc.tensor.matmul(
                        psumB, lhsT, rhs, start=(i == 0), stop=(i == 2 * KC - 1)
                    )
                scores = sc_pool.tile([P, E], f32)
                nc.scalar.activation(
                    out=scores,
                    in_=psumB,
                    func=mybir.ActivationFunctionType.Copy,
                    scale=1.0 / LO_SCALE,
                )
                nc.vector.tensor_add(scores, scores, psumA)
                _emit_topk(nc, sc_pool, out_pool, scores, wout, iout, b * TB + j * P)
    nc.compile()
    return nc


def _get_program(precision):
    key = f"nc_{precision}"
    if key not in _CACHE:
        _CACHE[key] = (
            _build_fp16_fp8dr() if precision == "fp16_fp8dr" else _build_fp16x3()
        )
    return _CACHE[key]


def _split_f16(a):
    hi = a.astype(np.float16)
    lo = ((a - hi.astype(np.float32)) * np.float32(LO_SCALE)).astype(np.float16)
    return hi, lo


def _prep_fp16_fp8dr(xt_full, wt_host):
    """Host-side operand prep for the fp16+fp8DR kernel.

    xt_full: [D, T_FULL] f32 (x transposed); wt_host: [D, E] f32.
    Returns per-core input dicts.  xl8 ships k-paired/blocked
    [NB, KC//2, P, 2, TB] so both src and dst DMA runs are 512B; x8 is
    cast from xh on device.
    """
    import ml_dtypes

    f8 = ml_dtypes.float8_e4m3
    S = np.float32(LO8_SCALE)
    xh_full = xt_full.astype(np.float16)                       # [D, T_FULL]
    xl8_full = ((xt_full - xh_full.astype(np.float32)) * S).astype(f8)
    wh_host = wt_host.astype(np.float16)                       # [D, E]
    wh8 = wh_host.astype(f8)
    wl8 = ((wt_host - wh_host.astype(np.float32)) * S).astype(f8)
    wi8_host = np.ascontiguousarray(np.stack([wh8, wl8], axis=1))  # [D, 2, E]

    in_maps = []
    for c in range(N_CORES):
        sl = slice(c * T, (c + 1) * T)
        # [D, T] -> [KC//2, 2, P, NB, TB] -> [NB, KC//2, P, 2, TB]
        xl8_c = xl8_full[:, sl].reshape(KC // 2, 2, P, NB, TB)
        xl8_c = np.ascontiguousarray(xl8_c.transpose(3, 0, 2, 1, 4))
        in_maps.append(
            {
                "xh": np.ascontiguousarray(xh_full[:, sl]),
                "xl8": xl8_c,
                "wh": wh_host,
                "wi8": wi8_host,
            }
        )
    return in_maps


def kernel(x: np.ndarray, weight: np.ndarray, _trace: bool = False, **_kw):
    x = np.asarray(x, dtype=np.float32)
    weight = np.asarray(weight, dtype=np.float32)
    assert x.shape == (T_FULL, D) and weight.shape == (E, D)

    nc = _get_program(PRECISION)
    xt_full = np.ascontiguousarray(x.T)              # [D, T_FULL]
    wt_host = np.ascontiguousarray(weight.T)         # [D, E]
    if PRECISION == "fp16_fp8dr":
        in_maps = _prep_fp16_fp8dr(xt_full, wt_host)
    else:
        xh_full, xl_full = _split_f16(xt_full)
        wh_host, wl_host = _split_f16(wt_host)
        in_maps = [
            {
                "xh": np.ascontiguousarray(xh_full[:, c * T:(c + 1) * T]),
                "xl": np.ascontiguousarray(xl_full[:, c * T:(c + 1) * T]),
                "wh": wh_host,
                "wl": wl_host,
            }
            for c in range(N_CORES)
        ]
    if _trace:
        import prof

        results, exec_time_ns, percore, neff_dir = prof.profiled_run(
            nc, in_maps, core_ids=list(range(N_CORES))
        )
        _CACHE["last_result"] = {
            "exec_time_ns": exec_time_ns,
            "percore": percore,
            "neff_dir": neff_dir,
        }
    else:
        res = run_bass_kernel_spmd(nc, in_maps, core_ids=list(range(N_CORES)))
        results = res.results
    w_full = np.concatenate([results[c]["w_out"] for c in range(N_CORES)], axis=0)
    i_full = np.concatenate(
        [results[c]["i_out"].astype(np.int32) for c in range(N_CORES)], axis=0
    )
    return w_full, i_full


# revision 18
# speedup vs baseline: 1.1679x; 1.0147x over previous
"""MoE group-limited routing gate (DeepSeek-style) on 8 Trainium2 NeuronCores.

Computation (per token t over E=256 experts, D=7168 features):
    logits = x @ weight.T                      [T, E]
    group-limited top-k: 8 groups of 32 experts, keep top-4 groups by
    group-max, then top-8 experts among kept groups.
    weights = sigmoid(logits[sel]) normalized to sum 1, * 2.5
Returns (weights [T,8] f32, indices [T,8] int32) like the reference.

Strategy: data-parallel over tokens, 2048 tokens/core, gate weight
replicated.  x is pre-transposed on host to [D, T] so the contraction dim
lands on SBUF partitions.  Matmul precision options:
  - "fp16_fp8dr" (default): logits = xh@wh (fp16 full-rate pass) +
    2^-15 * (xl8@wh8 + x8@wl8) where the two small cross terms are packed
    into a SINGLE fp8 DoubleRow pass (the PE computes
    lhsT[:,0].T@rhs[:,0] + lhsT[:,1].T@rhs[:,1] per matmul).  The cross
    terms are each ~2^-12 of the logit, so 4-bit-mantissa fp8 operands
    keep total logit error ~2e-5 (idx rel-err ~6e-3, CPU-validated).
    2.0 fp16-pass-equivalents of PE time vs fp16x3's 3.0.
  - "fp16x3": x and w split into fp16 (hi, lo*2^11) pairs;
    logits = hi@hi + 2^-11*(hi@lo2 + lo2@hi).  Exact to ~1e-6 but 3
    full-rate passes.
Top-k uses the DVE native max/max_index (top-8 sorted) instructions; the
group top-4 uses a threshold trick (4th-largest group-max) since sigmoid
is monotone and masking is additive on logits.
"""

import numpy as np
from contextlib import ExitStack

import concourse.bacc as bacc
import concourse.tile as tile
from concourse import mybir
from concourse.bass_utils import run_bass_kernel_spmd

N_CORES = 8
T_FULL = 16384
D = 7168
E = 256
G = 8            # expert groups
EPG = E // G     # experts per group = 32
TOPK = 8
TOPK_GROUPS = 4
ROUTE_SCALE = 2.5

P = 128
T = T_FULL // N_CORES       # 2048 tokens per core
KC = D // P                 # 56 contraction chunks
TB = 256                    # tokens per block
NB = T // TB                # 8 blocks
TPB = TB // P               # 2 token-tiles per block
KQ = 4                      # x DMA splits per block (finer-grained deps)
KCQ = KC // KQ              # 14 k-chunks per split
WQ = 8                      # weight DMA splits
WCQ = KC // WQ              # 7 k-chunks per split
NEG = -1.0e30
LO_SCALE = 2.0 ** 11        # fp16x3: host scales the fp16 lo term by this
LO8_SCALE = 2.0 ** 15       # fp16_fp8dr: host scales both fp8 lo terms by this
PRECISION = "fp16_fp8dr"    # "fp16_fp8dr" | "fp16x3"

_CACHE = {}


def _emit_topk(nc, sc_pool, out_pool, scores, wout, iout, t0, acc=None):
    """Group-limited top-k + normalize on a [128, 256] f32 logits tile.

    If acc=(wacc, iacc, ti) is given, results land in SBUF accumulators at
    tile index ti (batched output DMA at the end); otherwise each tile DMAs
    its own [128,8] results (32 tiny SWDGE writes — ~1us tail each)."""
    f32 = mybir.dt.float32
    scores_g = scores.rearrange("p (g e) -> p g e", g=G)
    glog = sc_pool.tile([P, G], f32)
    nc.vector.reduce_max(out=glog, in_=scores_g, axis=mybir.AxisListType.X)
    gsort = sc_pool.tile([P, G], f32)
    nc.vector.max(out=gsort, in_=glog)
    # additive mask: 0 for kept groups (>= 4th-largest), -1e30 otherwise
    maskadd = sc_pool.tile([P, G], f32)
    nc.vector.tensor_scalar(
        out=maskadd,
        in0=glog,
        scalar1=gsort[:, TOPK_GROUPS - 1:TOPK_GROUPS],
        scalar2=NEG,
        op0=mybir.AluOpType.is_lt,
        op1=mybir.AluOpType.mult,
    )
    masked = sc_pool.tile([P, E], f32)
    nc.vector.tensor_add(
        masked.rearrange("p (g e) -> p g e", g=G),
        scores_g,
        maskadd.to_broadcast([P, G, EPG]),
    )
    top8 = sc_pool.tile([P, TOPK], f32)
    nc.vector.max(out=top8, in_=masked)
    if acc is not None:
        wacc, iacc, ti = acc
        idx_out = iacc[:, ti, :]
        w_out_ap = wacc[:, ti, :]
    else:
        idx_out = out_pool.tile([P, TOPK], mybir.dt.uint32)
        w_out_ap = out_pool.tile([P, TOPK], f32)
    nc.vector.max_index(out=idx_out, in_max=top8, in_values=masked)
    sig = sc_pool.tile([P, TOPK], f32)
    nc.scalar.activation(
        out=sig, in_=top8, func=mybir.ActivationFunctionType.Sigmoid
    )
    ssum = sc_pool.tile([P, 1], f32)
    nc.vector.reduce_sum(out=ssum, in_=sig, axis=mybir.AxisListType.X)
    rec = sc_pool.tile([P, 1], f32)
    nc.vector.reciprocal(out=rec, in_=ssum)
    nc.vector.tensor_scalar(
        out=w_out_ap,
        in0=sig,
        scalar1=rec[:, 0:1],
        scalar2=ROUTE_SCALE,
        op0=mybir.AluOpType.mult,
        op1=mybir.AluOpType.mult,
    )
    if acc is None:
        # outputs ride the SWDGE ring so the tiny writes never stall the
        # HWDGE ring that streams x
        nc.gpsimd.dma_start(out=wout[t0:t0 + P, :], in_=w_out_ap)
        nc.gpsimd.dma_start(out=iout[t0:t0 + P, :], in_=idx_out)


def _build_fp16_fp8dr():
    """Main fp16 pass + single fp8 DoubleRow pass for both cross terms.

    Per 128-token tile: psumA = xh@wh (56 fp16 matmuls); psumB =
    2^15*(xl@wh + x@wl) (56 DoubleRow fp8 matmuls, each contracting one
    128-feature chunk for BOTH packed terms); scores = psumA + 2^-15*psumB.

    HBM traffic is 3B per x element: xh (fp16) + xl8 (fp8); the third
    operand x8 = fp8(xh) is cast on the otherwise-idle Pool engine into
    the second half of the DoubleRow lhsT tile.

    DRAM layouts are chosen so every DMA's contiguous run is exactly 512B
    (runs <512B pay a 2x DMA penalty): xh [D,T] f16 block slice -> 256*2B;
    xl8 k-paired [NB,KC/2,P,2,TB] -> (2,256)*1B both src and dst; wh
    [D,E] -> 512B; wi8 [D,2,E] -> 512B.
    """
    nc = bacc.Bacc("TRN2", target_bir_lowering=False, debug=False, num_devices=N_CORES)
    f32 = mybir.dt.float32
    f16 = mybir.dt.float16
    f8 = mybir.dt.float8e4
    xh = nc.dram_tensor("xh", [D, T], f16, kind="ExternalInput").ap()
    xl8 = nc.dram_tensor("xl8", [NB, KC // 2, P, 2, TB], f8, kind="ExternalInput").ap()
    wh = nc.dram_tensor("wh", [D, E], f16, kind="ExternalInput").ap()
    wi8 = nc.dram_tensor("wi8", [D, 2, E], f8, kind="ExternalInput").ap()
    wout = nc.dram_tensor("w_out", [T, TOPK], f32, kind="ExternalOutput").ap()
    iout = nc.dram_tensor("i_out", [T, TOPK], mybir.dt.uint32, kind="ExternalOutput").ap()

    xh_r = xh.rearrange("(k p) t -> p k t", p=P)
    # [p, nb, pair, two, t] view of the k-paired layout (k = pair*2 + two)
    xl8_r = xl8.rearrange("nb pair p two t -> p nb pair two t")
    wh_r = wh.rearrange("(k p) e -> p k e", p=P)
    wi8_r = wi8.rearrange("(k p) two e -> p k two e", p=P)

    with tile.TileContext(nc) as tc, ExitStack() as ctx:
        wt_pool = ctx.enter_context(tc.tile_pool(name="wt", bufs=1))
        xt_pool = ctx.enter_context(tc.tile_pool(name="xt", bufs=2))
        psA_pool = ctx.enter_context(tc.tile_pool(name="psA", bufs=4, space="PSUM"))
        psB_pool = ctx.enter_context(tc.tile_pool(name="psB", bufs=4, space="PSUM"))
        sc_pool = ctx.enter_context(tc.tile_pool(name="scratch", bufs=3))
        out_pool = ctx.enter_context(tc.tile_pool(name="outs", bufs=1))

        NT = NB * TPB  # 16 token tiles per core
        wacc = out_pool.tile([P, NT, TOPK], f32, tag="wacc")
        iacc = out_pool.tile([P, NT, TOPK], mybir.dt.uint32, tag="iacc")

        wh_sb, wi8_sb = [], []

        def load_wh(q):
            wtile = wt_pool.tile([P, WCQ, E], f16, tag=f"wh{q}")
            nc.sync.dma_start(out=wtile, in_=wh_r[:, q * WCQ:(q + 1) * WCQ, :])
            wh_sb.append(wtile)

        def load_wi8(q):
            wtile = wt_pool.tile([P, WCQ, 2, E], f8, tag=f"wi8{q}")
            nc.sync.dma_start(out=wtile, in_=wi8_r[:, q * WCQ:(q + 1) * WCQ, :, :])
            wi8_sb.append(wtile)

        def load_xh_q(b, q):
            xtile = xt_pool.tile([P, KCQ, TB], f16, tag=f"xh{q}")
            nc.sync.dma_start(
                out=xtile, in_=xh_r[:, q * KCQ:(q + 1) * KCQ, b * TB:(b + 1) * TB]
            )
            return xtile

        def load_xi8_q(b, q, xh_tile):
            """xi8 tile [P, 2, KCQ, TB]: [:,0]=xl8 (DMA), [:,1]=fp8(xh)
            (engine cast — keeps 14.7MB/core off the HBM stream).  The cast
            is byte-paced (~5us per quarter), so quarters alternate between
            the Pool and DVE engines to stay off the critical path."""
            xtile = xt_pool.tile([P, 2, KCQ, TB], f8, tag=f"xi8{q}")
            npair = KCQ // 2
            nc.sync.dma_start(
                out=xtile[:, 0, :, :].rearrange("p (pair two) t -> p pair two t", two=2),
                in_=xl8_r[:, b, q * npair:(q + 1) * npair, :, :],
            )
            eng = nc.gpsimd if (b * KQ + q) % 2 == 0 else nc.vector
            eng.tensor_scalar(
                out=xtile[:, 1, :, :],
                in0=xh_tile,
                scalar1=1.0,
                scalar2=None,
                op0=mybir.AluOpType.mult,
            )
            return xtile

        def load_block(b):
            xh_q, xi8_q = [], []
            for q in range(KQ):
                xh_q.append(load_xh_q(b, q))
            for q in range(KQ):
                xi8_q.append(load_xi8_q(b, q, xh_q[q]))
            return xh_q, xi8_q

        # DMA emission order == HWDGE arrival order (FIFO ring).  The B
        # (DoubleRow) pass of block b runs one block late, so the PE chews
        # A-passes while the wi8 + xl8 streams are still arriving:
        #   phase 1: wh eighths + xh0 quarters (first pieces split small so
        #            the first matmul issues ~2us in)
        #   phase 2: xh1 quarters
        #   phase 3: wi8 eighths + xl80 quarters
        #   then, per block b>=2: xh_b quarters, xl8_(b-1) quarters
        xh_blocks, xi8_blocks = {}, {}
        xh0 = []
        for q in range(KQ):
            load_wh(2 * q)
            if q == 0:
                # split the very first pieces so A0's k=0 deps land early
                xtile = xt_pool.tile([P, KCQ, TB], f16, tag="xh0")
                h = KCQ // 2
                nc.sync.dma_start(out=xtile[:, 0:h, :], in_=xh_r[:, 0:h, 0:TB])
                nc.sync.dma_start(out=xtile[:, h:KCQ, :], in_=xh_r[:, h:KCQ, 0:TB])
                xh0.append(xtile)
            else:
                xh0.append(load_xh_q(0, q))
            load_wh(2 * q + 1)
        xh_blocks[0] = xh0
        xh_blocks[1] = [load_xh_q(1, q) for q in range(KQ)]
        xi80 = []
        for q in range(KQ):
            load_wi8(2 * q)
            load_wi8(2 * q + 1)
            xi80.append(load_xi8_q(0, q, xh0[q]))
        xi8_blocks[0] = xi80

        def flush_b(b):
            """Emit block b's B-passes + score combine + topk."""
            xi8_q = xi8_blocks.pop(b)
            for j in range(TPB):
                js = slice(j * P, (j + 1) * P)
                psumA = psA_held.pop(0)
                psumB = psB_pool.tile([P, E], f32)
                for k in range(KC):
                    nc.tensor.matmul(
                        psumB,
                        xi8_q[k // KCQ][:, :, k % KCQ, js],
                        wi8_sb[k // WCQ][:, k % WCQ, :, :],
                        start=(k == 0),
                        stop=(k == KC - 1),
                        perf_mode=mybir.MatmulPerfMode.DoubleRow,
                    )
                scores = sc_pool.tile([P, E], f32)
                nc.scalar.activation(
                    out=scores,
                    in_=psumB,
                    func=mybir.ActivationFunctionType.Copy,
                    scale=1.0 / LO8_SCALE,
                )
                nc.vector.tensor_add(scores, scores, psumA)
                _emit_topk(
                    nc, sc_pool, out_pool, scores, wout, iout, b * TB + j * P,
                    acc=(wacc, iacc, b * TPB + j),
                )

        psA_held = []
        for b in range(NB):
            if b not in xh_blocks:
                xh_blocks[b] = [load_xh_q(b, q) for q in range(KQ)]
            if b - 1 >= 0 and (b - 1) not in xi8_blocks:
                xi8_blocks[b - 1] = [
                    load_xi8_q(b - 1, q, xh_prev[q]) for q in range(KQ)
                ]
            xh_q = xh_blocks.pop(b)
            for j in range(TPB):
                js = slice(j * P, (j + 1) * P)
                psumA = psA_pool.tile([P, E], f32)
                for k in range(KC):
                    nc.tensor.matmul(
                        psumA,
                        xh_q[k // KCQ][:, k % KCQ, js],
                        wh_sb[k // WCQ][:, k % WCQ, :],
                        start=(k == 0),
                        stop=(k == KC - 1),
                    )
                psA_held.append(psumA)
            if b >= 1:
                flush_b(b - 1)
            xh_prev = xh_q
        xi8_blocks[NB - 1] = [load_xi8_q(NB - 1, q, xh_prev[q]) for q in range(KQ)]
        flush_b(NB - 1)

        # batched output writes instead of 32 tiny per-tile DMAs, split so
        # the bulk overlaps the last block; they ride the SP/ACT HWDGE
        # rings so the two preps run in parallel (the SWDGE descriptor
        # build is ~1.7us each on Pool, serialized)
        wout_r = wout.rearrange("(i p) k -> p i k", p=P)
        iout_r = iout.rearrange("(i p) k -> p i k", p=P)
        NT_HEAD = NT - 4
        nc.sync.dma_start(out=wout_r[:, 0:NT_HEAD, :], in_=wacc[:, 0:NT_HEAD, :])
        nc.scalar.dma_start(out=iout_r[:, 0:NT_HEAD, :], in_=iacc[:, 0:NT_HEAD, :])
        nc.sync.dma_start(out=wout_r[:, NT_HEAD:NT, :], in_=wacc[:, NT_HEAD:NT, :])
        nc.scalar.dma_start(out=iout_r[:, NT_HEAD:NT, :], in_=iacc[:, NT_HEAD:NT, :])
    nc.compile()
    return nc


def _build_fp16x3():
    nc = bacc.Bacc("TRN2", target_bir_lowering=False, debug=False, num_devices=N_CORES)
    f32 = mybir.dt.float32
    f16 = mybir.dt.float16
    xh = nc.dram_tensor("xh", [D, T], f16, kind="ExternalInput").ap()
    xl = nc.dram_tensor("xl", [D, T], f16, kind="ExternalInput").ap()
    wh = nc.dram_tensor("wh", [D, E], f16, kind="ExternalInput").ap()
    wl = nc.dram_tensor("wl", [D, E], f16, kind="ExternalInput").ap()
    wout = nc.dram_tensor("w_out", [T, TOPK], f32, kind="ExternalOutput").ap()
    iout = nc.dram_tensor("i_out", [T, TOPK], mybir.dt.uint32, kind="ExternalOutput").ap()

    xh_r = xh.rearrange("(k p) t -> p k t", p=P)
    xl_r = xl.rearrange("(k p) t -> p k t", p=P)
    wh_r = wh.rearrange("(k p) e -> p k e", p=P)
    wl_r = wl.rearrange("(k p) e -> p k e", p=P)

    with tile.TileContext(nc) as tc, ExitStack() as ctx:
        wt_pool = ctx.enter_context(tc.tile_pool(name="wt", bufs=1))
        xt_pool = ctx.enter_context(tc.tile_pool(name="xt", bufs=2))
        # 4+4 slots = all 8 PSUM banks: block b's accumulators coexist with
        # block b-1's (whose xh@wl half is deferred one block, see below)
        psA_pool = ctx.enter_context(tc.tile_pool(name="psA", bufs=4, space="PSUM"))
        psB_pool = ctx.enter_context(tc.tile_pool(name="psB", bufs=4, space="PSUM"))
        sc_pool = ctx.enter_context(tc.tile_pool(name="scratch", bufs=3))
        out_pool = ctx.enter_context(tc.tile_pool(name="outs", bufs=4))

        def load_w(q, which):
            src, lst, tag = (
                (wh_r, wh_sb, f"wh{q}") if which == "h" else (wl_r, wl_sb, f"wl{q}")
            )
            wtile = wt_pool.tile([P, WCQ, E], f16, tag=tag)
            nc.sync.dma_start(out=wtile, in_=src[:, q * WCQ:(q + 1) * WCQ, :])
            lst.append(wtile)

        def load_x_block(b):
            xh_q, xl_q = [], []
            t_lo, t_hi = b * TB, (b + 1) * TB
            for q in range(KQ):
                xtile = xt_pool.tile([P, KCQ, TB], f16, tag=f"xh{q}")
                nc.sync.dma_start(
                    out=xtile, in_=xh_r[:, q * KCQ:(q + 1) * KCQ, t_lo:t_hi]
                )
                xh_q.append(xtile)
                ltile = xt_pool.tile([P, KCQ, TB], f16, tag=f"xl{q}")
                nc.sync.dma_start(
                    out=ltile, in_=xl_r[:, q * KCQ:(q + 1) * KCQ, t_lo:t_hi]
                )
                xl_q.append(ltile)
            return xh_q, xl_q

        wh_sb, wl_sb = [], []
        xh0, xl0 = [], []
        t_hi0 = TB
        for q in range(KQ):
            load_w(2 * q, "h")
            load_w(2 * q + 1, "h")
            xtile = xt_pool.tile([P, KCQ, TB], f16, tag=f"xh{q}")
            nc.sync.dma_start(out=xtile, in_=xh_r[:, q * KCQ:(q + 1) * KCQ, 0:t_hi0])
            xh0.append(xtile)
        for q in range(KQ):
            ltile = xt_pool.tile([P, KCQ, TB], f16, tag=f"xl{q}")
            nc.sync.dma_start(out=ltile, in_=xl_r[:, q * KCQ:(q + 1) * KCQ, 0:t_hi0])
            xl0.append(ltile)
        for q in range(WQ):
            load_w(q, "l")
        blocks = {0: (xh0, xl0)}

        def flush(state):
            bb, xh_q, psA_list, psB_list = state
            for j in range(TPB):
                js = slice(j * P, (j + 1) * P)
                psumB = psB_list[j]
                for k in range(KC):
                    nc.tensor.matmul(
                        psumB,
                        xh_q[k // KCQ][:, k % KCQ, js],
                        wl_sb[k // WCQ][:, k % WCQ, :],
                        start=False,
                        stop=(k == KC - 1),
                    )
                scores = sc_pool.tile([P, E], f32)
                nc.scalar.activation(
                    out=scores,
                    in_=psumB,
                    func=mybir.ActivationFunctionType.Copy,
                    scale=1.0 / LO_SCALE,
                )
                nc.vector.tensor_add(scores, scores, psA_list[j])
                _emit_topk(nc, sc_pool, out_pool, scores, wout, iout, bb * TB + j * P)

        pending = None
        for b in range(NB):
            if b not in blocks:
                blocks[b] = load_x_block(b)
            xh_q, xl_q = blocks.pop(b)
            if b == 0:
                psA_list, psB_list = [], []
                for j in range(TPB):
                    js = slice(j * P, (j + 1) * P)
                    psumA = psA_pool.tile([P, E], f32)
                    for k in range(KC):
                        nc.tensor.matmul(
                            psumA,
                            xh_q[k // KCQ][:, k % KCQ, js],
                            wh_sb[k // WCQ][:, k % WCQ, :],
                            start=(k == 0),
                            stop=(k == KC - 1),
                        )
                    psA_list.append(psumA)
                for j in range(TPB):
                    js = slice(j * P, (j + 1) * P)
                    psumB = psB_pool.tile([P, E], f32)
                    for k in range(KC):
                        nc.tensor.matmul(
                            psumB,
                            xl_q[k // KCQ][:, k % KCQ, js],
                            wh_sb[k // WCQ][:, k % WCQ, :],
                            start=(k == 0),
                            stop=False,
                        )
                    psB_list.append(psumB)
                pending = (b, xh_q, psA_list, psB_list)
                continue
            for j in range(TPB):
                js = slice(j * P, (j + 1) * P)
                psumA = psA_pool.tile([P, E], f32)
                for k in range(KC):
                    nc.tensor.matmul(
                        psumA,
                        xh_q[k // KCQ][:, k % KCQ, js],
                        wh_sb[k // WCQ][:, k % WCQ, :],
                        start=(k == 0),
                        stop=(k == KC - 1),
                    )
                if pending is not None:
                    flush(pending)
                    pending = None
                psumB = psB_pool.tile([P, E], f32)
                for i in range(2 * KC):
                    k = i % KC
                    if i < KC:
                        lhsT = xl_q[k // KCQ][:, k % KCQ, js]
                        rhs = wh_sb[k // WCQ][:, k % WCQ, :]
                    else:
                        lhsT = xh_q[k // KCQ][:, k % KCQ, js]
                        rhs = wl_sb[k // WCQ][:, k % WCQ, :]
                    nc.tensor.matmul(
                        psumB, lhsT, rhs, start=(i == 0), stop=(i == 2 * KC - 1)
                    )
                scores = sc_pool.tile([P, E], f32)
                nc.scalar.activation(
                    out=scores,
                    in_=psumB,
                    func=mybir.ActivationFunctionType.Copy,
                    scale=1.0 / LO_SCALE,
                )
                nc.vector.tensor_add(scores, scores, psumA)
                _emit_topk(nc, sc_pool, out_pool, scores, wout, iout, b * TB + j * P)
    nc.compile()
    return nc


def _get_program(precision):
    key = f"nc_{precision}"
    if key not in _CACHE:
        _CACHE[key] = (
            _build_fp16_fp8dr() if precision == "fp16_fp8dr" else _build_fp16x3()
        )
    return _CACHE[key]


def _split_f16(a):
    hi = a.astype(np.float16)
    lo = ((a - hi.astype(np.float32)) * np.float32(LO_SCALE)).astype(np.float16)
    return hi, lo


def _prep_fp16_fp8dr(xt_full, wt_host):
    """Host-side operand prep for the fp16+fp8DR kernel.

    xt_full: [D, T_FULL] f32 (x transposed); wt_host: [D, E] f32.
    Returns per-core input dicts.  xl8 ships k-paired/blocked
    [NB, KC//2, P, 2, TB] so both src and dst DMA runs are 512B; x8 is
    cast from xh on device.
    """
    import ml_dtypes

    f8 = ml_dtypes.float8_e4m3
    S = np.float32(LO8_SCALE)
    xh_full = xt_full.astype(np.float16)                       # [D, T_FULL]
    xl8_full = ((xt_full - xh_full.astype(np.float32)) * S).astype(f8)
    wh_host = wt_host.astype(np.float16)                       # [D, E]
    wh8 = wh_host.astype(f8)
    wl8 = ((wt_host - wh_host.astype(np.float32)) * S).astype(f8)
    wi8_host = np.ascontiguousarray(np.stack([wh8, wl8], axis=1))  # [D, 2, E]

    in_maps = []
    for c in range(N_CORES):
        sl = slice(c * T, (c + 1) * T)
        # [D, T] -> [KC//2, 2, P, NB, TB] -> [NB, KC//2, P, 2, TB]
        xl8_c = xl8_full[:, sl].reshape(KC // 2, 2, P, NB, TB)
        xl8_c = np.ascontiguousarray(xl8_c.transpose(3, 0, 2, 1, 4))
        in_maps.append(
            {
                "xh": np.ascontiguousarray(xh_full[:, sl]),
                "xl8": xl8_c,
                "wh": wh_host,
                "wi8": wi8_host,
            }
        )
    return in_maps


def kernel(x: np.ndarray, weight: np.ndarray, _trace: bool = False, **_kw):
    x = np.asarray(x, dtype=np.float32)
    weight = np.asarray(weight, dtype=np.float32)
    assert x.shape == (T_FULL, D) and weight.shape == (E, D)

    nc = _get_program(PRECISION)
    xt_full = np.ascontiguousarray(x.T)              # [D, T_FULL]
    wt_host = np.ascontiguousarray(weight.T)         # [D, E]
    if PRECISION == "fp16_fp8dr":
        in_maps = _prep_fp16_fp8dr(xt_full, wt_host)
    else:
        xh_full, xl_full = _split_f16(xt_full)
        wh_host, wl_host = _split_f16(wt_host)
        in_maps = [
            {
                "xh": np.ascontiguousarray(xh_full[:, c * T:(c + 1) * T]),
                "xl": np.ascontiguousarray(xl_full[:, c * T:(c + 1) * T]),
                "wh": wh_host,
                "wl": wl_host,
            }
            for c in range(N_CORES)
        ]
    if _trace:
        import prof

        results, exec_time_ns, percore, neff_dir = prof.profiled_run(
            nc, in_maps, core_ids=list(range(N_CORES))
        )
        _CACHE["last_result"] = {
            "exec_time_ns": exec_time_ns,
            "percore": percore,
            "neff_dir": neff_dir,
        }
    else:
        res = run_bass_kernel_spmd(nc, in_maps, core_ids=list(range(N_CORES)))
        results = res.results
    w_full = np.concatenate([results[c]["w_out"] for c in range(N_CORES)], axis=0)
    i_full = np.concatenate(
        [results[c]["i_out"].astype(np.int32) for c in range(N_CORES)], axis=0
    )
    return w_full, i_full
